# revision 7
# baseline (speedup 1.0000x reference)
"""Trainium2 Bass kernel for nn_DeepRecursiveNetwork (v2: delta-PSUM design).

Math (reference): 30 outer steps; each step, per block n (0..9):
    inp  = h[n] + block_in[n]           (block_in = x_emb for n=0 else h[n-1] from prev step)
    inner equilibrium, 5 iters from h'=0:
        h' = 0.5 h' + 0.5 tanh(h' @ W[n].T + b[n] + inp)
    h[n] = 0.5 h[n] + 0.5 h'
Output: h[9] @ head_W.T + head_b.

Device formulation (per core, 8-way data parallel over batch, B_local=128):
  All recurrent tensors live TRANSPOSED in SBUF as [128, 8*128] f16 tiles laid
  out (d_lo, (d_hi, b)); matmuls (out = lhsT.T @ rhs, contraction on the
  partition dim) need no transposes anywhere.

  Substitution u_j = 2*h'_j with pre-halved weights Wh = W.T/2:
      u_{j+1} = 0.5 u_j + t_j,   t_j = tanh(Wh.T u_j + c),  c = v[n] + bin
      u_1 = t_0 = tanh(c),       v[n] <- 0.5 v[n] + 0.25 u_5

  Delta accumulation: the pre-activation s_j = Wh.T u_j + c lives in PSUM for
  the whole inner loop.  The j1 matmul group opens each bank (start=True)
  computing W*u_1; c is then added into PSUM in place by DVE per half-bank
  (DVE writes don't touch the has_written bits the matmuls set, so later
  groups keep accumulating).  Each further iteration accumulates only
  W @ du where du_{j+1} = u_{j+1} - u_j = t_j - 0.5 u_j, k-major so the
  first 32 matmuls need only the lo half of du.  t_0 = tanh(c) reads SBUF;
  t_j reads PSUM as two wide [128,512] tanh instructions, each followed
  immediately by its du half (tensor_tensor SUB, f16 2x mode) so the next
  matmul group starts one Act-half earlier.  th = 0.5*u uses tensor_scalar
  (4x mode).  Inner-loop tensors are f16; the outer state v is f32 (the
  30-step accumulator dominates rounding error) with a fused 2-op f32 tail
  v' = 0.25*t4 + (0.25*th4 + 0.5*v) on DVE; c = v+bin and vh = 0.5*v run
  on the idle GpSimd/Pool engine (they have >= 1 pair of lead time).

  Matmuls in fp16 (full PE rate); two complementary fp16 roundings of the
  weights on alternating outer steps cancel correlated rounding bias.

  Exact cone skipping: block n stays exactly 0 until step n (the signal
  propagates one block per step from x_emb), and is DEAD (cannot influence
  the output head) once n < s - (steps - NB).  Step s therefore only
  processes the trapezoid max(0, s-(steps-NB)) <= n <= min(s, NB-1):
  210 of 300 block-steps for steps=30, both cuts bit-exact.  Units are
  paired consecutively across step boundaries (flat wavefront pairing)
  with a validity check (a unit whose prologue reads a block its pair
  partner writes becomes a singleton).

  Schedule: the next pair's c/vh run at body top (Pool), its t_0 tanh is
  emitted after the current pair's chains (so it queues on Act behind the
  critical t_j reads), gated by a reads-vs-writes hoist check.  PSUM is
  split into two pair-slots (4 banks each) used alternately.  Weights (one
  2 MB fp16 DMA per block) are prefetched one pair ahead into a 4-deep
  buffer ring.  Measured (TimelineSim): 2.920 ms, PE ~98% busy at the
  fp16 roofline for the live trapezoid.
"""

import numpy as np

import concourse.bass as bass
import concourse.bacc as bacc
import concourse.mybir as mybir
from concourse.bass_utils import run_bass_kernel_spmd
from concourse.tile import TileContext

F32 = mybir.dt.float32
F16 = mybir.dt.float16

B, DIN, H, DOUT, NB = 1024, 512, 1024, 512, 10
NCORES = 8
BL = B // NCORES  # 128 batch per core
KH = H // 128     # 8 k/m tiles over H
KD = DIN // 128   # 4 k tiles over DIN
KO = DOUT // 128  # 4 m tiles over DOUT
INNER = 5
Tanh = mybir.ActivationFunctionType.Tanh
Ident = mybir.ActivationFunctionType.Identity
MULT = mybir.AluOpType.mult
ADD = mybir.AluOpType.add
SUB = mybir.AluOpType.subtract


def build_nc(steps: int, has_b: bool = False):
    nc = bacc.Bacc(None, target_bir_lowering=False)
    xT = nc.dram_tensor("xT", [128, KD * BL], F16, kind="ExternalInput")
    embWT = nc.dram_tensor("embWT", [128, KD * H], F16, kind="ExternalInput")
    embB = nc.dram_tensor("embB", [128, KH], F32, kind="ExternalInput")
    Wab = nc.dram_tensor("Wab", [2, NB, 128, KH * H], F16, kind="ExternalInput")
    headWT = nc.dram_tensor("headWT", [128, KH * DOUT], F16, kind="ExternalInput")
    headB = nc.dram_tensor("headB", [128, KO], F32, kind="ExternalInput")
    if has_b:
        bK1D = nc.dram_tensor("bK1", [1, NB * KH * 128], F16, kind="ExternalInput")
        onesD = nc.dram_tensor("ones16", [1, 128], F16, kind="ExternalInput")
        bTD = nc.dram_tensor("bT", [128, NB * KH], F32, kind="ExternalInput")
    outT = nc.dram_tensor("outT", [128, KO * BL], F32, kind="ExternalOutput")

    NPAIR = steps * 5

    with TileContext(nc) as tc:
        with (
            tc.tile_pool(name="const", bufs=1) as cpool,
            tc.tile_pool(name="state", bufs=1) as spool,
            tc.tile_pool(name="wts", bufs=4) as wpool,
            tc.tile_pool(name="work", bufs=2) as kpool,
            tc.tile_pool(name="psum", bufs=1, space="PSUM") as ppool,
        ):
            # ---- constants (staged through DVE copies: downstream consumers
            # then depend on a single DVE queue position) ----
            xT_sb0 = cpool.tile([128, KD * BL], F16, tag="xt0", bufs=1)
            embWT_sb0 = cpool.tile([128, KD * H], F16, tag="embwt0", bufs=1)
            embB_sb0 = cpool.tile([128, KH], F32, tag="embb0", bufs=1)
            headWT_sb0 = cpool.tile([128, KH * DOUT], F16, tag="hwt0", bufs=1)
            headB_sb0 = cpool.tile([128, KO], F32, tag="hb0", bufs=1)
            # xT/embWT go on the sync (SP) queue FIRST: the DMA pipeline is
            # serial, and the 2 MB weight transfers (also on sync, emitted
            # later) must not delay the embed inputs needed at t~2us
            nc.sync.dma_start(xT_sb0[:], xT[:])
            nc.sync.dma_start(embWT_sb0[:], embWT[:])
            nc.gpsimd.dma_start(embB_sb0[:], embB[:])
            nc.gpsimd.dma_start(headWT_sb0[:], headWT[:])
            nc.gpsimd.dma_start(headB_sb0[:], headB[:])
            if has_b:
                bK1_sb0 = cpool.tile([1, NB * KH * 128], F16, tag="bk0", bufs=1)
                ones_sb0 = cpool.tile([1, 128], F16, tag="on0", bufs=1)
                bT_sb0 = cpool.tile([128, NB * KH], F32, tag="bt0", bufs=1)
                nc.gpsimd.dma_start(bK1_sb0[:], bK1D[:])
                nc.gpsimd.dma_start(ones_sb0[:], onesD[:])
                nc.gpsimd.dma_start(bT_sb0[:], bTD[:])

            headWT_sb = cpool.tile([128, KH * DOUT], F16, tag="hwt", bufs=1)
            headB_sb = cpool.tile([128, KO], F32, tag="hb", bufs=1)
            stage = [(headWT_sb, headWT_sb0), (headB_sb, headB_sb0)]
            xT_sb, embWT_sb, embB_sb = xT_sb0, embWT_sb0, embB_sb0
            if has_b:
                bK1_sb = cpool.tile([1, NB * KH * 128], F16, tag="bk", bufs=1)
                ones_sb = cpool.tile([1, 128], F16, tag="on", bufs=1)
                bT_sb = cpool.tile([128, NB * KH], F32, tag="bt", bufs=1)
                stage += [(bK1_sb, bK1_sb0), (ones_sb, ones_sb0),
                          (bT_sb, bT_sb0)]
            for dst, srcv in stage:
                nc.vector.tensor_copy(dst[:], srcv[:])

            # ---- persistent state ----
            # v in f32: the 30-step outer accumulator is the dominant
            # rounding-error source; f32 here halves final error at zero
            # makespan cost (the f32 DVE ops are off the critical chain).
            v = [spool.tile([128, H], F32, tag=f"v{n}", bufs=1, name=f"v{n}")
                 for n in range(NB)]
            xemb = spool.tile([128, H], F32, tag="xemb", bufs=1)
            # v[0] first: the bootstrap prologue (pair 0 = block 0) only
            # needs v[0]; the rest can drain behind it
            nc.vector.memset(v[0][:], 0.0)
            for n in range(1, NB):
                nc.vector.memset(v[n][:], 0.0)

            # ---- PSUM: [pair-slot][pair-member][half-bank] ----
            ps = [[[ppool.tile([128, 512], F32, tag=f"ps{s}{a}{h}", bufs=1,
                               name=f"ps{s}{a}{h}")
                    for h in range(2)] for a in range(2)] for s in range(2)]

            # flat bank list: one bank per m-group for embed/head (a PSUM bank
            # supports only ONE accumulation group at a time; start=True
            # lazily zeroes the whole 2KB bank)
            flat_banks = [ps[s][a][h] for s in range(2) for a in range(2)
                          for h in range(2)]

            # ---- embed: xemb = (x @ embed_W.T + embed_b)^T, f16 matmuls ----
            for m in range(KH):
                bank = flat_banks[m]
                col = slice(0, 128)
                for k in range(KD):
                    nc.tensor.matmul(
                        bank[:, col],
                        embWT_sb[:, k * H + m * 128: k * H + (m + 1) * 128],
                        xT_sb[:, k * BL: (k + 1) * BL],
                        start=(k == 0),
                        stop=(k == KD - 1),
                    )
                nc.scalar.activation(
                    xemb[:, m * 128: (m + 1) * 128], bank[:, col], Ident,
                    bias=embB_sb[:, m: m + 1], scale=1.0,
                )

            # ---------------- main recurrence ----------------
            # Exact-zero skipping: block n stays exactly zero until step n
            # (signal propagates one block per step from x_emb), so step s
            # only processes blocks 0..min(s, NB-1), in descending order.
            # Units are paired consecutively ACROSS step boundaries (flat
            # wavefront pairing): a cross-step pair ((0,s), (m,s+1)) is
            # data-independent since m = min(s+1, NB-1) >= 2 for s >= 1, so
            # member B never reads v[0].  Only (0,0) must stay a singleton
            # ((1,1) reads v[0] which (0,0) writes).
            # Backward cone: the output is head_W @ v[NB-1] after the last
            # step, and influence propagates one block per step, so block n
            # at step s is DEAD (cannot reach the output) when
            # n < s - (steps - NB).  Together with the forward zero cone the
            # live region is a trapezoid: 210 of 300 units for steps=30.
            units = []  # (block, step)
            for s in range(steps):
                lo = max(0, s - (steps - NB))
                for n in range(min(s, NB - 1), lo - 1, -1):
                    units.append((n, s))
            # Greedy consecutive pairing with a validity check: member B's
            # prologue reads v[nB], v[nB-1]; it cannot share a pair with a
            # member A that writes one of those.
            pair_list = []  # tuple of (block, step) members, len 1 or 2
            i = 0
            while i < len(units):
                if i + 1 < len(units):
                    u1, u2 = units[i], units[i + 1]
                    if u1[0] in (u2[0], u2[0] - 1):
                        pair_list.append((u1,))
                        i += 1
                        continue
                    pair_list.append((u1, u2))
                    i += 2
                else:
                    pair_list.append((units[i],))
                    i += 1
            NP = len(pair_list)
            # v-blocks read by a pair's prologue (c = v[n] + v[n-1], vh = v[n]/2)
            reads = []
            for mem in pair_list:
                r = set()
                for n, s in mem:
                    r.add(n)
                    if n > 0:
                        r.add(n - 1)
                reads.append(r)

            def blocks_of(g):
                return [n for n, s in pair_list[g]]

            def ktile(tag, bufs):
                return kpool.tile([128, H], F16, tag=tag, bufs=bufs, name=tag)

            def dma_weights(g):
                # two half-transfers per block: k-major consumers can start
                # on the first 1 MB half while the second streams
                ws = []
                for n, s in pair_list[g]:
                    w = wpool.tile([128, KH * H], F16, tag="w", bufs=4, name="w")
                    half = KH * H // 2
                    nc.sync.dma_start(w[:, :half], Wab[s % 2, n, :, :half])
                    nc.sync.dma_start(w[:, half:], Wab[s % 2, n, :, half:])
                    ws.append(w)
                return ws

            def emit_c_vh(g):
                out = []
                for n in blocks_of(g):
                    bin_t = xemb if n == 0 else v[n - 1]
                    c = ktile("c", 3)
                    nc.gpsimd.tensor_add(c[:], v[n][:], bin_t[:])
                    vh = kpool.tile([128, H], F32, tag="vh", bufs=3, name="vh")
                    nc.gpsimd.tensor_scalar_mul(vh[:], v[n][:], 0.5)
                    out.append((c, vh))
                return out

            def emit_binject(g, ai, n):
                # has_b only: inject b[n] (broadcast over batch) via K=1
                # matmuls; these carry the bank group's start=True
                sl = g % 2
                for m in range(KH):
                    bank = ps[sl][ai][m // 4]
                    col = slice((m % 4) * 128, (m % 4 + 1) * 128)
                    o = (n * KH + m) * 128
                    nc.tensor.matmul(bank[:, col],
                                     bK1_sb[0:1, o: o + 128],
                                     ones_sb[0:1, :],
                                     start=(m % 4 == 0), stop=False,
                                     skip_group_check=True)

            def emit_t0(g, cvh):
                # t_0 = tanh(c [+ b]) straight from SBUF -- no PE involved
                out = []
                for ai, n in enumerate(blocks_of(g)):
                    c = cvh[ai][0]
                    t0 = ktile("t0", 3)
                    if has_b:
                        for m in range(KH):
                            mc = slice(m * 128, (m + 1) * 128)
                            nc.scalar.activation(
                                t0[:, mc], c[:, mc], Tanh,
                                bias=bT_sb[:, n * KH + m: n * KH + m + 1],
                                scale=1.0)
                    else:
                        nc.scalar.activation(t0[:, :512], c[:, :512], Tanh)
                        nc.scalar.activation(t0[:, 512:], c[:, 512:], Tanh)
                    th1 = ktile("th", 4)
                    nc.vector.tensor_scalar_mul(th1[:], t0[:], 0.5)
                    out.append((t0, th1))
                return out

            def emit_mm(g, ai, w, rhs, last):
                # k-major: the first 32 matmuls consume only rhs[:, :512]
                # (the lo half of du), which lands one Act-half earlier
                sl = g % 2
                for k in range(KH):
                    for m in range(KH):
                        bank = ps[sl][ai][m // 4]
                        col = slice((m % 4) * 128, (m % 4 + 1) * 128)
                        nc.tensor.matmul(
                            bank[:, col],
                            w[:, k * H + m * 128: k * H + (m + 1) * 128],
                            rhs[:, k * 128: (k + 1) * 128],
                            start=False,
                            stop=(last and k == KH - 1 and m % 4 == 3),
                            skip_group_check=True,
                        )

            def emit_j1(g, ai, w, rhs, c):
                # j1 opens each bank's accumulation group (start=True unless
                # the b-injection already did) and computes W*u_1; the c-add
                # into PSUM runs on DVE per half-bank, interleaved so the
                # lo-bank add overlaps the hi-bank matmuls.  DVE writes don't
                # touch the has_written bits set by the j1 matmuls, so j2+
                # keep accumulating on top.
                sl = g % 2
                for h in range(2):
                    bank = ps[sl][ai][h]
                    for m in range(h * 4, h * 4 + 4):
                        col = slice((m % 4) * 128, (m % 4 + 1) * 128)
                        for k in range(KH):
                            nc.tensor.matmul(
                                bank[:, col],
                                w[:, k * H + m * 128: k * H + (m + 1) * 128],
                                rhs[:, k * 128: (k + 1) * 128],
                                start=(not has_b and k == 0 and m % 4 == 0),
                                stop=False,
                                skip_group_check=True,
                            )
                    nc.vector.tensor_tensor(
                        bank[:], bank[:], c[:, h * 512: (h + 1) * 512], ADD)

            def emit_tj_chain(g, ai, th):
                """t_j tanh from PSUM + du/u/th updates.  du is computed per
                half right after its tanh half so the next k-major matmul
                group can start as soon as du-lo exists; the off-critical
                u-add runs on Pool to keep the DVE queue short."""
                sl = g % 2
                t = ktile("t", 4)
                du = ktile("du", 4)
                for h in range(2):
                    hs = slice(h * 512, (h + 1) * 512)
                    nc.scalar.activation(t[:, hs], ps[sl][ai][h][:], Tanh)
                    nc.vector.tensor_tensor(du[:, hs], t[:, hs], th[:, hs], SUB)
                u = ktile("u", 4)
                nc.vector.tensor_add(u[:], th[:], t[:])
                thn = ktile("th", 4)
                nc.vector.tensor_scalar_mul(thn[:], u[:], 0.5)
                return du, thn

            def emit_final(g, ai, n, th4, vh):
                sl = g % 2
                t4 = ktile("t", 4)
                nc.scalar.activation(t4[:, :512], ps[sl][ai][0][:], Tanh)
                nc.scalar.activation(t4[:, 512:], ps[sl][ai][1][:], Tanh)
                a = kpool.tile([128, H], F32, tag="a", bufs=2, name="a")
                nc.vector.scalar_tensor_tensor(a[:], th4[:], 0.25, vh[:], MULT, ADD)
                nc.vector.scalar_tensor_tensor(v[n][:], t4[:], 0.25, a[:], MULT, ADD)

            # bootstrap prologue for pair 0 (weight DMAs last: the embed
            # constants must win the DMA engines first)
            if NP:
                cvh = emit_c_vh(0)
                t0s = emit_t0(0, cvh)
                wts = dma_weights(0)

            for g in range(NP):
                blks = blocks_of(g)
                nm = len(blks)
                ws = wts
                cs = [cvh[ai][0] for ai in range(nm)]
                th = [t0s[ai][1] for ai in range(nm)]  # current th per member
                t0 = [t0s[ai][0] for ai in range(nm)]
                vh = [cvh[ai][1] for ai in range(nm)]
                more = g + 1 < NP
                # prologue for pair g+1 can be hoisted into this body iff
                # none of the v-blocks it reads are written by THIS pair
                hoist = more and not (set(blks) & reads[g + 1])
                if more:
                    wts = dma_weights(g + 1)
                    if hoist:
                        cvh = emit_c_vh(g + 1)

                rhs = t0
                for j in range(1, INNER):                   # j1..j4 groups
                    lastj = j == INNER - 1
                    nrhs = [None] * nm
                    for ai in range(nm):
                        if j == 1:
                            if has_b:
                                emit_binject(g, ai, blks[ai])
                            emit_j1(g, ai, ws[ai], rhs[ai], cs[ai])
                        else:
                            emit_mm(g, ai, ws[ai], rhs[ai], last=lastj)
                        if not lastj:
                            du, thn = emit_tj_chain(g, ai, th[ai])
                            nrhs[ai] = du
                            th[ai] = thn
                    rhs = nrhs

                # next pair's t0 emitted here: late enough that it queues on
                # Act behind this pair's critical t_j reads, early enough to
                # complete before the next pair's j1 needs it
                if more and hoist:
                    t0s = emit_t0(g + 1, cvh)

                for ai, n in enumerate(blks):
                    emit_final(g, ai, n, th[ai], vh[ai])

                if more and not hoist:
                    cvh = emit_c_vh(g + 1)
                    t0s = emit_t0(g + 1, cvh)

            # ---- head: out^T = head_W @ v[9]^T + head_b ----
            # v[9] is f32; matmul operands must both be f16 -> one copy
            v9h = ktile("v9h", 1)
            nc.vector.tensor_copy(v9h[:], v[NB - 1][:])
            outsb = kpool.tile([128, KO * BL], F32, tag="outsb", bufs=1)
            for mo in range(KO):
                bank = flat_banks[mo]
                col = slice(0, 128)
                for k in range(KH):
                    nc.tensor.matmul(
                        bank[:, col],
                        headWT_sb[:, k * DOUT + mo * 128: k * DOUT + (mo + 1) * 128],
                        v9h[:, k * 128: (k + 1) * 128],
                        start=(k == 0),
                        stop=(k == KH - 1),
                    )
                nc.scalar.activation(
                    outsb[:, mo * BL: (mo + 1) * BL], bank[:, col], Ident,
                    bias=headB_sb[:, mo: mo + 1], scale=1.0,
                )
            nc.sync.dma_start(outT[:], outsb[:])
    nc.compile()
    return nc


def _tile_k(a):
    """[K, M] -> [128, (K//128)*M] laid out (k_lo, (k_hi, m))."""
    K, M = a.shape
    return np.ascontiguousarray(
        a.reshape(K // 128, 128, M).transpose(1, 0, 2).reshape(128, (K // 128) * M)
    )


def prep_inputs(inputs):
    x = np.asarray(inputs["x"], np.float32)
    embed_W = np.asarray(inputs["embed_W"], np.float32)
    embed_b = np.asarray(inputs["embed_b"], np.float32)
    block_W = np.asarray(inputs["block_W"], np.float32)
    block_b = np.asarray(inputs["block_b"], np.float32)
    head_W = np.asarray(inputs["head_W"], np.float32)
    head_b = np.asarray(inputs["head_b"], np.float32)
    steps = int(np.asarray(inputs["steps"]))
    has_b = bool(np.any(block_b != 0.0))

    embWT = _tile_k(embed_W.T.astype(np.float16))
    headWT = _tile_k(head_W.T.astype(np.float16))
    Wt = block_W.transpose(0, 2, 1) * np.float32(0.5)  # [NB, K=h_in, M=d_out]
    Wa = Wt.astype(np.float16)
    Wb = (2.0 * Wt - Wa.astype(np.float32)).astype(np.float16)
    Wab = np.stack(
        [
            np.stack([_tile_k(Wa[n]) for n in range(NB)]),
            np.stack([_tile_k(Wb[n]) for n in range(NB)]),
        ]
    )  # [2, NB, 128, 8*1024] f16
    embB = np.ascontiguousarray(embed_b.reshape(KH, 128).T)
    headB = np.ascontiguousarray(head_b.reshape(KO, 128).T)
    common = dict(embWT=embWT, embB=embB, Wab=Wab, headWT=headWT,
                  headB=headB)
    if has_b:
        common["bK1"] = np.ascontiguousarray(
            block_b.reshape(1, NB * KH * 128).astype(np.float16))
        common["ones16"] = np.ones((1, 128), np.float16)
        common["bT"] = np.ascontiguousarray(
            block_b.reshape(NB, KH, 128).transpose(2, 0, 1
                ).reshape(128, NB * KH).astype(np.float32))

    in_maps = []
    for ci in range(NCORES):
        xT = _tile_k(
            np.ascontiguousarray(x[ci * BL: (ci + 1) * BL].T).astype(np.float16))
        m = dict(common)
        m["xT"] = xT
        in_maps.append(m)
    return in_maps, steps, has_b


def kernel(**inputs) -> np.ndarray:
    in_maps, steps, has_b = prep_inputs(inputs)
    nc = build_nc(steps, has_b)
    res = run_bass_kernel_spmd(nc, in_maps, core_ids=list(range(NCORES)))

    out = np.empty((B, DOUT), np.float32)
    for ci in range(NCORES):
        oT = res.results[ci]["outT"]  # [128, (do_hi=4, b=128)] = out^T tiled
        out[ci * BL: (ci + 1) * BL] = (
            oT.reshape(128, KO, BL).transpose(2, 1, 0).reshape(BL, DOUT)
        )
    return out
